# revision 27
# baseline (speedup 1.0000x reference)
"""EdgeConv (PyG, aggr='max') Trainium2 kernel, 8-core SPMD.

Math: out_i = max_{e: dst(e)=i} relu(x_i @ W1.T + (x_src(e) - x_i) @ W2.T + b)
with W = [W1 | W2].  Rewriting:
    msg_e = relu(A_i + g_src(e)),  A = x @ (W1-W2).T + b,  g = x @ W2.T
Since A_i is constant within segment i and relu is monotone:
    out_i = relu(A_i + max_e g_src(e))
The reference's dst is repeat(arange(N), DEG) (fixed-degree kNN-style graph),
so segments are 16 consecutive edges; segment-max becomes a grouped reduce.

Two SPMD launches on 8 cores:
  L1 (node-parallel): per-core 6250-node shard computes hT = wcat.T @ xT in
     channel-major orientation. The host pre-transposes x (bf16), so L1 is
     13 wide PE matmuls (512-column PSUM-bank tiles, no on-device
     transposes); A-channels get the bias via DVE, g-channels are copied to
     bf16 by ACT. Outputs are written channel-major contiguous.
  L2 (edge-parallel): per-core 100k-edge shard bulk-gathers 256B bf16
     row-PAIRS [g_{2r} | g_{2r+1}] by src>>1 with non-transpose dma_gather
     (one descriptor per edge; src>>1 <= 24999 fits int16; pad positions
     read the sentinel pair-row NPAIR = -3e38). Parity half-select uses a
     host-precomputed uint8 mask via copy_predicated (ACT copy + DVE
     predicated overwrite). A host-side edge permutation lands node n's 16
     slots at partition n%128, slots 16*(n//128)+k, so the segment max is a
     bf16 max-tree (packed 2x DVE mode). Results accumulate in SBUF (bf16)
     and are written in two halves; the host converts to f32.
"""

import numpy as np
import ml_dtypes

BF16 = ml_dtypes.bfloat16

N_NODES = 50000
DEG = 16
C = 64
N_CORES = 8
NSH = N_NODES // N_CORES  # 6250 nodes per core
P = 128
TCH = 2  # node-tiles per L2 chunk
CHUNK = TCH * P  # 256 nodes per L2 chunk
NSH_PAD = 6400  # 25 chunks * 256; 50 tiles * 128
NT = NSH_PAD // P  # 50
NCHUNKS = NSH_PAD // CHUNK  # 25
NI = CHUNK * DEG  # 4096 gather positions per chunk
NPAIR = N_NODES // 2  # 256B row-pairs in the gather table
SENT = -3.0e38
SUP = 512  # dense supertile columns (one PSUM bank)

_cache = {}


def _build_dense():
    import concourse.bacc as bacc
    import concourse.mybir as mybir
    from concourse.tile import TileContext

    nc = bacc.Bacc("TRN2", target_bir_lowering=False, debug=False)
    f32 = mybir.dt.float32
    bf16 = mybir.dt.bfloat16
    # xt: column n = x[shard_base+n], plus a trailing ones row (bias input)
    xt = nc.dram_tensor("xt", [C + 1, NSH_PAD], bf16, kind="ExternalInput")
    # wcat[in, 0:64] = (W1-W2).T ; wcat[in, 64:128] = W2.T ; row C = [b | 0]
    wcat = nc.dram_tensor("wcat", [C + 1, 2 * C], bf16, kind="ExternalInput")
    # gat[0:64] = A channels (bias included), gat[64:128] = g channels
    gat = nc.dram_tensor("gat", [2 * C, NSH_PAD], bf16, kind="ExternalOutput")

    nsup = NSH_PAD // SUP  # 12 supertiles of 512 + 1 of 256
    rem = NSH_PAD - nsup * SUP
    spans = [(i * SUP, SUP) for i in range(nsup)] + ([(nsup * SUP, rem)] if rem else [])

    with TileContext(nc) as tc:
        with (
            tc.tile_pool(name="const", bufs=1) as cpool,
            tc.tile_pool(name="sbuf", bufs=1) as pool,
            tc.tile_pool(name="psum", bufs=6, space="PSUM") as psum,
        ):
            w_sb = cpool.tile([C + 1, 2 * C], bf16)
            nc.sync.dma_start(out=w_sb[:], in_=wcat[:])
            # preload the ACT function table while xt streams in
            warm = cpool.tile([1, 2], f32)
            nc.vector.memset(warm[:], 0.0)
            warm2 = cpool.tile([1, 2], f32)
            nc.scalar.copy(out=warm2[:], in_=warm[:])
            xt_sb = pool.tile([C + 1, NSH_PAD], bf16, tag="xt")
            xsp = [(0, SUP), (SUP, 4 * SUP), (4 * SUP, 8 * SUP), (8 * SUP, NSH_PAD)]
            for k, (a, b) in enumerate(xsp):
                nc.sync.dma_start(out=xt_sb[:, a:b], in_=xt[:, a:b])
            # staged output tiles: each piece is written to HBM as soon as its
            # supertiles are done, overlapping the remaining compute
            pieces = [(0, 5), (5, 10), (10, 12), (12, len(spans))]
            ga_t = []
            for q0, q1 in pieces:
                w = spans[q1 - 1][0] + spans[q1 - 1][1] - spans[q0][0]
                ga_piece = pool.tile([2 * C, w], bf16, tag=f"ga{q0}", name=f"ga{q0}")
                ga_t.append(ga_piece)
            for i, (s0, sl) in enumerate(spans):
                cols = slice(s0, s0 + sl)
                ps = psum.tile([2 * C, SUP], f32, tag="h")
                nc.tensor.matmul(
                    out=ps[:, 0:sl], lhsT=w_sb[:], rhs=xt_sb[:, cols],
                    start=True, stop=True,
                )
                pi = next(j for j, (q0, q1) in enumerate(pieces) if q0 <= i < q1)
                base = spans[pieces[pi][0]][0]
                dst = ga_t[pi][:, s0 - base : s0 - base + sl]
                if i % 2 == 0:
                    nc.scalar.copy(out=dst, in_=ps[:, 0:sl])
                else:
                    nc.vector.tensor_copy(out=dst, in_=ps[:, 0:sl])
                if i == pieces[pi][1] - 1:
                    hi = spans[pieces[pi][1] - 1][0] + spans[pieces[pi][1] - 1][1]
                    nc.sync.dma_start(out=gat[:, base:hi], in_=ga_t[pi][:])
    nc.compile()
    return nc


def _build_gather():
    import concourse.bacc as bacc
    import concourse.mybir as mybir
    from concourse.tile import TileContext

    nc = bacc.Bacc("TRN2", target_bir_lowering=False, debug=False)
    f32 = mybir.dt.float32
    bf16 = mybir.dt.bfloat16
    i16 = mybir.dt.int16
    u8 = mybir.dt.uint8
    mx = mybir.AluOpType.max
    # pair table: row r = [g_{2r} | g_{2r+1}] (256B); row NPAIR = sentinel
    gpair = nc.dram_tensor("gpair", [NPAIR + 1, 2 * C], bf16, kind="ExternalInput")
    idx = nc.dram_tensor("idx", [P, NCHUNKS * (NI // 16)], i16, kind="ExternalInput")
    msk = nc.dram_tensor("msk", [P, NCHUNKS * TCH * DEG], u8, kind="ExternalInput")
    ash = nc.dram_tensor("ash", [P, NT * C], bf16, kind="ExternalInput")
    osh = nc.dram_tensor("osh", [P, (NT - 1) * C], bf16, kind="ExternalOutput")

    with TileContext(nc) as tc:
        with (
            tc.tile_pool(name="sbuf", bufs=1) as pool,
            tc.tile_pool(name="gat", bufs=4) as gpool,
        ):
            # idx split so chunk 0 can start gathering immediately; msk and
            # the second ash half slot into the DMA queue before/after the
            # long idx tail without delaying the first transfers
            S = NI // 16
    
            idx0 = pool.tile([P, 1, S], i16, tag="idx0")
            nc.sync.dma_start(
                out=idx0[:], in_=idx[:, 0:S].rearrange("p (h s) -> p h s", h=1)
            )
            idxA = pool.tile([P, 5, S], i16, tag="idxA")
            nc.sync.dma_start(
                out=idxA[:],
                in_=idx[:, S : 6 * S].rearrange("p (h s) -> p h s", h=5),
            )
            msk_all = pool.tile([P, NCHUNKS, TCH * DEG], u8, tag="msk")
            nc.sync.dma_start(
                out=msk_all[:], in_=msk[:].rearrange("p (h s) -> p h s", h=NCHUNKS)
            )
            a_all = pool.tile([P, NT, C], bf16, tag="a")
            nc.sync.dma_start(
                out=a_all[:, 0 : NT // 2, :],
                in_=ash[:, 0 : NT // 2 * C].rearrange("p (t c) -> p t c", t=NT // 2),
            )
            idxB = pool.tile([P, NCHUNKS - 6, S], i16, tag="idxB")
            o_all = pool.tile([P, NT - 1, C], bf16, tag="o")
            # last chunk is half-size: node-tile 49 (6272-6399) is all padding
            for ch in range(NCHUNKS):
                tch = TCH if ch < NCHUNKS - 1 else 1
                ni = tch * P * DEG
                # position j lands at partition j%128, slot j//128; each slot
                # holds a 256B row-pair [even | odd]
                gath = gpool.tile([P, TCH, DEG, 2, C], bf16, tag="gath")
                nc.gpsimd.dma_gather(
                    out_ap=gath[:, 0:tch, :, :, :].rearrange(
                        "p t k two c -> p (t k) (two c)"
                    ),
                    in_ap=gpair[:],
                    idxs_ap=(
                        idx0[:, 0, 0 : ni // 16]
                        if ch == 0
                        else idxA[:, ch - 1, 0 : ni // 16]
                        if ch < 6
                        else idxB[:, ch - 6, 0 : ni // 16]
                    ),
                    num_idxs=ni,
                    num_idxs_reg=ni,
                    elem_size=2 * C,
                    transpose=False,
                    queue_num=0,
                    single_packet=False,
                )
                if ch == 2:
                    nc.sync.dma_start(
                        out=idxB[:],
                        in_=idx[:, 6 * S :].rearrange(
                            "p (h s) -> p h s", h=NCHUNKS - 6
                        ),
                    )
                    nc.sync.dma_start(
                        out=a_all[:, NT // 2 :, :],
                        in_=ash[:, NT // 2 * C :].rearrange(
                            "p (t c) -> p t c", t=NT - NT // 2
                        ),
                    )
                # parity select in place: odd overwrites even where mask=1
                nc.vector.copy_predicated(
                    out=gath[:, 0:tch, :, 0, :],
                    mask=msk_all[:, ch, 0 : tch * DEG]
                    .rearrange("p (t k) -> p t k", k=DEG)
                    .to_broadcast([P, tch, DEG, C]),
                    data=gath[:, 0:tch, :, 1, :],
                )
                # segment max as a bf16 max-tree (packed free dim -> 2x DVE)
                t1 = gpool.tile([P, TCH, 8, C], bf16, tag="t1")
                nc.vector.tensor_tensor(
                    out=t1[:, 0:tch],
                    in0=gath[:, 0:tch, 0:8, 0, :],
                    in1=gath[:, 0:tch, 8:16, 0, :],
                    op=mx,
                )
                t2 = gpool.tile([P, TCH, 4, C], bf16, tag="t2")
                nc.vector.tensor_tensor(
                    out=t2[:, 0:tch],
                    in0=t1[:, 0:tch, 0:4, :],
                    in1=t1[:, 0:tch, 4:8, :],
                    op=mx,
                )
                t3 = gpool.tile([P, TCH, 2, C], bf16, tag="t3")
                nc.vector.tensor_tensor(
                    out=t3[:, 0:tch],
                    in0=t2[:, 0:tch, 0:2, :],
                    in1=t2[:, 0:tch, 2:4, :],
                    op=mx,
                )
                m_sb = gpool.tile([P, TCH, C], f32, tag="m")
                nc.vector.tensor_tensor(
                    out=m_sb[:, 0:tch],
                    in0=t3[:, 0:tch, 0, :],
                    in1=t3[:, 0:tch, 1, :],
                    op=mx,
                )
                s_sb = gpool.tile([P, TCH, C], f32, tag="s")
                nc.vector.tensor_add(
                    out=s_sb[:, 0:tch],
                    in0=m_sb[:, 0:tch],
                    in1=a_all[:, ch * TCH : ch * TCH + tch, :],
                )
                nc.scalar.activation(
                    out=o_all[:, ch * TCH : ch * TCH + tch, :],
                    in_=s_sb[:, 0:tch],
                    func=mybir.ActivationFunctionType.Relu,
                )
                if ch in (11, 19, 23):
                    lo = {11: 0, 19: 24, 23: 40}[ch]
                    hi = (ch + 1) * TCH
                    nc.sync.dma_start(
                        out=osh[:, lo * C : hi * C].rearrange(
                            "p (t c) -> p t c", t=hi - lo
                        ),
                        in_=o_all[:, lo:hi, :],
                    )
            nc.sync.dma_start(
                out=osh[:, 48 * C : (NT - 1) * C].rearrange(
                    "p (t c) -> p t c", t=NT - 1 - 48
                ),
                in_=o_all[:, 48:, :],
            )
    nc.compile()
    return nc


def _make_indices(src_pad):
    """src_pad: [NSH_PAD, DEG] int64 node ids (pad rows = -1).
    Returns (idx, msk): pair-row indices (src>>1, sentinel NPAIR for pads) in
    dma_gather's index layout (16 partitions), and the odd-parity mask in dest
    layout [128, slots]. Position j of chunk ch covers node n_c = j%128 +
    128*(j//128 // DEG) ... specifically j = (DEG*(n_c//128)+k)*128 + (n_c%128)."""
    s = src_pad.reshape(NCHUNKS, TCH, P, DEG)
    flat = np.transpose(s, (0, 1, 3, 2)).reshape(NCHUNKS, NI)  # [ch, (t k p)]
    pidx = np.where(flat >= 0, flat >> 1, NPAIR).astype(np.int16)
    par = np.where(flat >= 0, flat & 1, 0).astype(np.uint8)
    # index layout: position j -> [j%16, j//16], replicated 8x down partitions
    a = np.swapaxes(pidx.reshape(NCHUNKS, NI // 16, 16), 1, 2)  # [ch, 16, s]
    idx = np.ascontiguousarray(
        np.tile(a, (1, 8, 1)).transpose(1, 0, 2).reshape(P, NCHUNKS * (NI // 16))
    )
    # mask layout: dest [partition j%128, slot j//128]
    m = np.swapaxes(par.reshape(NCHUNKS, TCH * DEG, P), 1, 2)
    msk = np.ascontiguousarray(m.transpose(1, 0, 2).reshape(P, NCHUNKS * TCH * DEG))
    return idx, msk


def _numpy_fallback(x, edge_index, W, b):
    src, dst = edge_index[0], edge_index[1]
    V1 = W[:, :C] - W[:, C:]
    V2 = W[:, C:]
    A = x @ V1.T + b
    g = x @ V2.T
    out = np.full((x.shape[0], C), -np.inf, dtype=np.float32)
    msg = np.maximum(A[dst] + g[src], 0.0)
    np.maximum.at(out, dst, msg)
    return np.where(np.isneginf(out), 0.0, out).astype(np.float32)


def _run_spmd(nc, in_maps):
    # the shared axon device occasionally reports a transient
    # NRT_EXEC_UNIT_UNRECOVERABLE on a cold first launch; retry once
    import time
    from concourse.bass_utils import run_bass_kernel_spmd

    try:
        return run_bass_kernel_spmd(nc, in_maps, core_ids=list(range(N_CORES)))
    except Exception:
        time.sleep(10.0)
        return run_bass_kernel_spmd(nc, in_maps, core_ids=list(range(N_CORES)))


def kernel(x, edge_index, edge_attr, W, b):

    x = np.ascontiguousarray(x, dtype=np.float32)
    edge_index = np.ascontiguousarray(edge_index, dtype=np.int32)
    W = np.ascontiguousarray(W, dtype=np.float32)
    b = np.ascontiguousarray(b, dtype=np.float32)

    expected_dst = np.repeat(np.arange(N_NODES, dtype=np.int32), DEG)
    if (
        x.shape != (N_NODES, C)
        or edge_index.shape != (2, N_NODES * DEG)
        or not np.array_equal(edge_index[1], expected_dst)
    ):
        return _numpy_fallback(x, edge_index, W, b)

    if "dense" not in _cache:
        _cache["dense"] = _build_dense()
    if "gather" not in _cache:
        _cache["gather"] = _build_gather()

    # ---- Launch 1: node-parallel dense phase (channel-major) ----
    # wcat[in, 0:64] = (W1-W2).T, wcat[in, 64:128] = W2.T, row C = [b | 0]
    W1, W2 = W[:, :C], W[:, C:]
    wcat = np.concatenate(
        [
            np.concatenate([(W1 - W2).T, W2.T], axis=1),
            np.concatenate([b, np.zeros(C, np.float32)]).reshape(1, 2 * C),
        ],
        axis=0,
    ).astype(BF16)
    xtp = np.zeros((C + 1, N_CORES, NSH_PAD), dtype=BF16)
    xtp[:C, :, :NSH] = (
        x.astype(BF16).reshape(N_CORES, NSH, C).transpose(2, 0, 1)
    )
    xtp[C] = 1.0
    in1 = [
        {"xt": np.ascontiguousarray(xtp[:, c, :]), "wcat": wcat}
        for c in range(N_CORES)
    ]
    r1 = _run_spmd(_cache["dense"], in1)

    # host: assemble the full pair table [g_2r | g_2r+1] + sentinel row
    g_full = np.concatenate(
        [r1.results[c]["gat"][C:, :NSH].T for c in range(N_CORES)], axis=0
    )  # [N_NODES, C] bf16
    gpair = np.concatenate(
        [
            np.ascontiguousarray(g_full).reshape(NPAIR, 2 * C),
            np.full((1, 2 * C), SENT, dtype=BF16),
        ],
        axis=0,
    )
    gpair = np.ascontiguousarray(gpair)

    # ---- Launch 2: edge-parallel gather + segment max ----
    src = edge_index[0]
    in2 = []
    for c in range(N_CORES):
        s = np.full((NSH_PAD, DEG), -1, dtype=np.int64)
        s[:NSH] = src[c * NSH * DEG : (c + 1) * NSH * DEG].reshape(NSH, DEG)
        idx, msk = _make_indices(s)
        # ash[p, t*C+c] = A[128*t + p, c] of this core's shard
        at = r1.results[c]["gat"][:C]  # [C, NSH_PAD] bf16
        ash = np.ascontiguousarray(
            at.T.reshape(NT, P, C).transpose(1, 0, 2).reshape(P, NT * C)
        )
        in2.append({"gpair": gpair, "idx": idx, "msk": msk, "ash": ash})
    r2 = _run_spmd(_cache["gather"], in2)

    out = np.empty((N_CORES, NSH, C), dtype=np.float32)
    for c in range(N_CORES):
        o = (
            r2.results[c]["osh"]
            .reshape(P, NT - 1, C)
            .transpose(1, 0, 2)
            .reshape((NT - 1) * P, C)
        )
        out[c] = o[:NSH].astype(np.float32)
    _cache["last_results"] = (r1, r2)
    return out.reshape(N_NODES, C)
